# revision 1
# baseline (speedup 1.0000x reference)
"""Trainium2 Bass kernel for segment_reduce (max over groups of min within group).

reference semantics:
    mins = min(x[:, groups], axis=-1)   # [B, G]
    out  = max(mins, axis=1, keepdims=True)  # [B, 1]

Strategy:
  - Pure data parallel: 8 NeuronCores, each handles B/8 = 131072 rows.
  - Host side: transpose x to feature-major [32, B] so each feature's column
    loads as a unit-stride SBUF tile [128, 1024] (rows spread over
    partition x free dims). DMA casts fp32 -> fp16 in flight (SWDGE).
  - Device side: the whole reduction is an elementwise min/max dataflow over
    32 feature tiles. The group structure is known at compile time (groups is
    tiny), so we bake a deduplicated op schedule: drop dominated groups,
    reuse shared feature pairs, then one max-accumulate chain.
    fp16 tensor_tensor runs in the DVE 2x perf mode; min/max are selections
    so the only rounding is the initial fp32->fp16 cast (rel err <= 2^-11).
  - Store: DMA casts fp16 -> fp32 on the way out.
"""

import itertools
import os
from collections import Counter

import numpy as np

_B, _N, _G = 1048576, 32, 64
_NC = 8
_R = _B // _NC  # rows per core
_P = 128
_W = _R // _P  # free-dim width per feature tile
_FCHUNK = 4  # features per load DMA

_DT_NAME = os.environ.get("KERNEL_DT", "float16")

_prog_cache: dict = {}


def _plan(groups: np.ndarray):
    """Compile-time schedule: dedup groups, factor shared pairs.

    Returns (kept, choice) where kept is a list of sorted tuples of distinct
    feature ids (len 1-3) and choice maps kept-index -> the pair used for its
    first min (only for len-3 groups).
    """
    gs = [tuple(sorted(set(int(v) for v in g))) for g in groups]
    sets = [frozenset(g) for g in gs]
    kept = []
    for i, si in enumerate(sets):
        dominated = False
        for j, sj in enumerate(sets):
            if i == j:
                continue
            # if another group's feature set is a proper subset, its min is
            # >= ours, so we can never win the max -> drop us. Equal sets:
            # keep only the first.
            if sj < si or (sj == si and j < i):
                dominated = True
                break
        if not dominated:
            kept.append(gs[i])

    def pairs_of(g):
        return [tuple(sorted(p)) for p in itertools.combinations(g, 2)]

    computed = set(g for g in kept if len(g) == 2)
    choice = {}
    unresolved = [i for i, g in enumerate(kept) if len(g) == 3]
    for i in list(unresolved):
        for pr in pairs_of(kept[i]):
            if pr in computed:
                choice[i] = pr
                unresolved.remove(i)
                break
    while unresolved:
        cnt = Counter(pr for i in unresolved for pr in pairs_of(kept[i]))
        best = max(cnt, key=lambda p: (cnt[p], -max(p)))
        computed.add(best)
        for i in list(unresolved):
            if best in pairs_of(kept[i]):
                choice[i] = best
                unresolved.remove(i)
    return kept, choice


def _build_blocks(kept, choice):
    """Order work so each computed pair's users are adjacent (short pair
    lifetimes) and feature chunks are consumed roughly in DMA order."""
    pair_users: dict = {}
    singles = []
    for i, g in enumerate(kept):
        if len(g) == 1:
            singles.append(i)
        elif len(g) == 2:
            pair_users.setdefault(g, []).append((i, None))
        else:
            pr = choice[i]
            third = (set(g) - set(pr)).pop()
            pair_users.setdefault(pr, []).append((i, third))
    blocks = []
    for pr, users in pair_users.items():
        users = sorted(users, key=lambda u: (-1 if u[1] is None else u[1]))
        blocks.append(("pair", pr, users))
    for i in singles:
        blocks.append(("single", kept[i][0], None))
    # earliest feature chunk whose arrival unblocks the block's first op
    def block_key(b):
        if b[0] == "pair":
            return (max(b[1]), b[1])
        return (b[1], (b[1],))

    blocks.sort(key=block_key)
    return blocks


def _build_program(groups_tuple, dt_name):
    from concourse import bacc, mybir
    from concourse.tile import TileContext

    groups = np.array(groups_tuple, dtype=np.int64)
    kept, choice = _plan(groups)
    blocks = _build_blocks(kept, choice)

    DT = getattr(mybir.dt, dt_name)
    cast = dt_name != "float32"

    nc = bacc.Bacc("TRN2", debug=False, enable_asserts=False, num_devices=_NC)
    xt = nc.dram_tensor("xt", [_N, _R], mybir.dt.float32, kind="ExternalInput")
    out = nc.dram_tensor("out", [_R], mybir.dt.float32, kind="ExternalOutput")

    n_tt = 0
    with TileContext(nc) as tc:
        with (
            tc.tile_pool(name="feats", bufs=1) as fpool,
            tc.tile_pool(name="pairs", bufs=6) as ppool,
            tc.tile_pool(name="work", bufs=4) as wpool,
        ):
            nchunk = _N // _FCHUNK
            chunks = []
            for ci in range(nchunk):
                t = fpool.tile([_P, _FCHUNK, _W], DT, tag=f"chunk{ci}")
                dma = nc.gpsimd if cast else nc.sync
                dma.dma_start(
                    out=t[:],
                    in_=xt[ci * _FCHUNK : (ci + 1) * _FCHUNK, :].rearrange(
                        "f (p w) -> p f w", p=_P
                    ),
                )
                chunks.append(t)

            def feat(f):
                return chunks[f // _FCHUNK][:, f % _FCHUNK, :]

            M = wpool.tile([_P, _W], DT, tag="acc")
            first = True
            for b in blocks:
                if b[0] == "single":
                    if first:
                        nc.vector.tensor_copy(out=M[:], in_=feat(b[1]))
                        first = False
                    else:
                        nc.vector.tensor_tensor(
                            M[:], M[:], feat(b[1]), mybir.AluOpType.max
                        )
                        n_tt += 1
                    continue
                _, pr, users = b
                pt = ppool.tile([_P, _W], DT, tag="pair")
                nc.vector.tensor_tensor(
                    pt[:], feat(pr[0]), feat(pr[1]), mybir.AluOpType.min
                )
                n_tt += 1
                for _i, third in users:
                    if third is None:
                        contrib = pt[:]
                    elif first:
                        nc.vector.tensor_tensor(
                            M[:], pt[:], feat(third), mybir.AluOpType.min
                        )
                        n_tt += 1
                        first = False
                        continue
                    else:
                        u = wpool.tile([_P, _W], DT, tag="u")
                        nc.vector.tensor_tensor(
                            u[:], pt[:], feat(third), mybir.AluOpType.min
                        )
                        n_tt += 1
                        contrib = u[:]
                    if first:
                        nc.vector.tensor_copy(out=M[:], in_=contrib)
                        first = False
                    else:
                        nc.vector.tensor_tensor(
                            M[:], M[:], contrib, mybir.AluOpType.max
                        )
                        n_tt += 1

            st = nc.gpsimd if cast else nc.sync
            st.dma_start(out=out.rearrange("(p w) -> p w", p=_P), in_=M[:])

    nc.compile()
    stats = {
        "n_groups_kept": len(kept),
        "n_pairs": sum(1 for b in blocks if b[0] == "pair"),
        "n_tt_ops": n_tt,
    }
    return nc, stats


def _get_program(groups: np.ndarray):
    key = (tuple(map(tuple, np.asarray(groups).tolist())), _DT_NAME)
    if key not in _prog_cache:
        _prog_cache[key] = _build_program(key[0], _DT_NAME)
    return _prog_cache[key]


def run(x, groups, trace=False):
    """Returns (out [B,1] fp32, BassKernelResults)."""
    from concourse import bass_utils

    x = np.asarray(x)
    groups = np.asarray(groups)
    assert x.shape == (_B, _N), x.shape
    nc, stats = _get_program(groups)

    xt = np.ascontiguousarray(x.T.astype(np.float32, copy=False))  # [32, B]
    in_maps = [
        {"xt": np.ascontiguousarray(xt[:, c * _R : (c + 1) * _R])} for c in range(_NC)
    ]
    res = bass_utils.run_bass_kernel_spmd(
        nc, in_maps, core_ids=list(range(_NC)), trace=trace
    )
    y = (
        np.concatenate([np.asarray(res.results[c]["out"]) for c in range(_NC)])
        .astype(np.float32)
        .reshape(_B, 1)
    )
    return y, res, stats


def kernel(x, groups):
    y, _res, _stats = run(x, groups, trace=False)
    return y


# revision 4
# speedup vs baseline: 1.3516x; 1.3516x over previous
"""Trainium2 Bass kernel for segment_reduce (max over groups of min within group).

reference semantics:
    mins = min(x[:, groups], axis=-1)   # [B, G]
    out  = max(mins, axis=1, keepdims=True)  # [B, 1]

Strategy:
  - Pure data parallel: 8 NeuronCores, each handles B/8 = 131072 rows.
  - Host side: transpose x to feature-major [32, B] so each feature's column
    loads as a unit-stride SBUF tile [128, 1024] (rows spread over
    partition x free dims). DMA casts fp32 -> fp16 in flight (SWDGE).
  - Device side: the whole reduction is an elementwise min/max dataflow over
    32 feature tiles. The group structure is known at compile time (groups is
    tiny), so we bake an optimized op schedule:
      * drop dominated groups (superset feature-sets can never win the max)
      * factor via the distributive lattice law
            max_i min(f, A_i) = min(f, max_i A_i)
        building a 2-level trie (pivot f -> second pivot g -> max over third
        features), which cuts tensor ops to ~(G-1) + #f-nodes + #fg-nodes.
    All rewrites are exact in the (min,max) lattice. fp16 tensor_tensor runs
    in the DVE 2x perf mode; min/max are selections so the only rounding is
    the initial fp32->fp16 cast (rel err <= 2^-11).
  - The host layout also permutes features into first-use order so compute
    can start as soon as the first DMA chunks land.
  - Store: DMA casts fp16 -> fp32 on the way out.
"""

import os
from collections import Counter, OrderedDict

import numpy as np

_B, _N, _G = 1048576, 32, 64
_NC = 8
_R = _B // _NC  # rows per core
_P = 128
_W = _R // _P  # free-dim width per feature tile
_FCHUNK = 2  # features per load DMA

_DT_NAME = os.environ.get("KERNEL_DT", "float16")

_prog_cache: dict = {}


def _dedup(groups: np.ndarray):
    """Sorted-distinct tuples, dominated groups removed."""
    gs = [tuple(sorted(set(int(v) for v in g))) for g in groups]
    sets = [frozenset(g) for g in gs]
    kept = []
    for i, si in enumerate(sets):
        dominated = False
        for j, sj in enumerate(sets):
            if i == j:
                continue
            # a proper-subset group has min >= ours, so ours never wins the max
            if sj < si or (sj == si and j < i):
                dominated = True
                break
        if not dominated:
            kept.append(gs[i])
    return kept


def _build_trie(kept):
    """Greedy 2-level factoring.

    Returns (trie, singles) where trie is an ordered dict
        f -> OrderedDict(g -> [thirds...])
    covering all groups of size >= 2, and singles is a list of size-1 groups'
    features. Op count = (#contribs-1) root maxes + sum over f of
    (1 min + (#children-1) maxes) + sum over fg with thirds of
    (1 min + (#thirds-1) maxes).
    """
    singles = [g[0] for g in kept if len(g) == 1]
    todo = [g for g in kept if len(g) >= 2]
    trie = OrderedDict()
    while todo:
        cnt = Counter(f for g in todo for f in g)
        f = max(cnt, key=lambda k: (cnt[k], -k))
        cluster = [g for g in todo if f in g]
        todo = [g for g in todo if f not in g]
        rests = [tuple(v for v in g if v != f) for g in cluster]
        sub = OrderedDict()
        while rests:
            c2 = Counter(v for r in rests for v in r)
            g2 = max(c2, key=lambda k: (c2[k], -k))
            mine = [r for r in rests if g2 in r]
            rests = [r for r in rests if g2 not in r]
            thirds = []
            for r in mine:
                rem = tuple(v for v in r if v != g2)
                if rem:
                    thirds.append(rem[0])
                else:
                    thirds.append(None)  # 2-group: value is just feat(g2)
            sub[g2] = thirds
        trie[f] = sub
    return trie, singles


def _schedule_ops(trie, singles):
    """Flatten the trie into an op list over virtual values.

    Ops: ("min"|"max", dst, src0, src1) where srcs are ("feat", f) or
    ("buf", id); plus ("copy", dst, src) fallback. dst is ("buf", id) or
    ("acc",). Returns (ops, n_tt).
    """
    ops = []
    buf_id = [0]

    def newbuf():
        buf_id[0] += 1
        return ("buf", buf_id[0])

    acc = ("acc",)
    acc_started = False

    for f, sub in trie.items():
        v_acc = None  # cluster accumulator value
        for g2, thirds in sub.items():
            real = [t for t in thirds if t is not None]
            if real and len(real) != len(thirds):
                # {f,g2} subset of {f,g2,c} means the 3-group was dominated
                # and dropped, so a None can't coexist with real thirds.
                raise AssertionError("dominated group survived dedup")
            if real:
                if len(real) == 1:
                    u = ("feat", real[0])
                else:
                    ub = newbuf()
                    ops.append(("max", ub, ("feat", real[0]), ("feat", real[1])))
                    for t3 in real[2:]:
                        ops.append(("max", ub, ub, ("feat", t3)))
                    u = ub
                if u[0] == "buf":
                    ops.append(("min", u, ("feat", g2), u))
                    t = u
                else:
                    tb = newbuf()
                    ops.append(("min", tb, ("feat", g2), u))
                    t = tb
            else:
                t = ("feat", g2)  # pure 2-group {f, g2}
            if v_acc is None:
                v_acc = t
            elif v_acc[0] == "buf":
                ops.append(("max", v_acc, v_acc, t))
            else:
                vb = newbuf()
                ops.append(("max", vb, v_acc, t))
                v_acc = vb
        # min with the pivot, folded straight into the root accumulator so no
        # cluster result outlives its cluster (keeps pool pressure low)
        if not acc_started:
            ops.append(("min", acc, ("feat", f), v_acc))
            acc_started = True
        else:
            if v_acc[0] == "buf":
                ops.append(("min", v_acc, ("feat", f), v_acc))
                ops.append(("max", acc, acc, v_acc))
            else:
                cb = newbuf()
                ops.append(("min", cb, ("feat", f), v_acc))
                ops.append(("max", acc, acc, cb))

    for s in singles:
        if not acc_started:
            ops.append(("copy", acc, ("feat", s)))
            acc_started = True
        else:
            ops.append(("max", acc, acc, ("feat", s)))
    n_tt = sum(1 for o in ops if o[0] in ("min", "max"))
    return ops, n_tt


def _feature_order(ops):
    """Permutation: feature ids in first-use order (unused features last)."""
    order = []
    seen = set()
    for op in ops:
        for s in op[2:]:
            if isinstance(s, tuple) and s[0] == "feat" and s[1] not in seen:
                seen.add(s[1])
                order.append(s[1])
    for f in range(_N):
        if f not in seen:
            order.append(f)
    return order


def _build_program(groups_tuple, dt_name):
    from concourse import bacc, mybir
    from concourse.tile import TileContext

    groups = np.array(groups_tuple, dtype=np.int64)
    kept = _dedup(groups)
    trie, singles = _build_trie(kept)
    ops, n_tt = _schedule_ops(trie, singles)
    forder = _feature_order(ops)
    fpos = {f: i for i, f in enumerate(forder)}  # feature id -> storage slot

    DT = getattr(mybir.dt, dt_name)
    cast = dt_name != "float32"

    nc = bacc.Bacc("TRN2", debug=False, enable_asserts=False, num_devices=_NC)
    xt = nc.dram_tensor("xt", [_N, _R], mybir.dt.float32, kind="ExternalInput")
    out = nc.dram_tensor("out", [_R], mybir.dt.float32, kind="ExternalOutput")

    with TileContext(nc) as tc:
        with (
            tc.tile_pool(name="feats", bufs=1) as fpool,
            tc.tile_pool(name="work", bufs=8) as wpool,
        ):
            nchunk = _N // _FCHUNK
            chunks = []
            for ci in range(nchunk):
                t = fpool.tile([_P, _FCHUNK, _W], DT, tag=f"chunk{ci}")
                dma = nc.gpsimd if cast else nc.sync
                dma.dma_start(
                    out=t[:],
                    in_=xt[ci * _FCHUNK : (ci + 1) * _FCHUNK, :].rearrange(
                        "f (p w) -> p f w", p=_P
                    ),
                )
                chunks.append(t)

            def feat_ap(f):
                s = fpos[f]
                return chunks[s // _FCHUNK][:, s % _FCHUNK, :]

            buf_tiles = {}
            acc_tile = wpool.tile([_P, _W], DT, tag="acc")

            def val_ap(v):
                if v[0] == "feat":
                    return feat_ap(v[1])
                if v[0] == "acc":
                    return acc_tile[:]
                return buf_tiles[v][:]

            def dst_ap(v):
                if v[0] == "acc":
                    return acc_tile[:]
                if v not in buf_tiles:
                    buf_tiles[v] = wpool.tile(
                        [_P, _W], DT, tag="u", name=f"u{v[1]}"
                    )
                return buf_tiles[v][:]

            alu = {"min": mybir.AluOpType.min, "max": mybir.AluOpType.max}
            for op in ops:
                if op[0] == "copy":
                    nc.vector.tensor_copy(out=dst_ap(op[1]), in_=val_ap(op[2]))
                else:
                    nc.vector.tensor_tensor(
                        dst_ap(op[1]), val_ap(op[2]), val_ap(op[3]), alu[op[0]]
                    )

            st = nc.gpsimd if cast else nc.sync
            st.dma_start(out=out.rearrange("(p w) -> p w", p=_P), in_=acc_tile[:])

    nc.compile()
    stats = {
        "n_groups_kept": len(kept),
        "n_f_nodes": len(trie),
        "n_fg_nodes": sum(len(s) for s in trie.values()),
        "n_tt_ops": n_tt,
    }
    return nc, forder, stats


def _get_program(groups: np.ndarray):
    key = (tuple(map(tuple, np.asarray(groups).tolist())), _DT_NAME)
    if key not in _prog_cache:
        _prog_cache[key] = _build_program(key[0], _DT_NAME)
    return _prog_cache[key]


def run(x, groups, trace=False):
    """Returns (out [B,1] fp32, BassKernelResults, stats)."""
    from concourse import bass_utils

    x = np.asarray(x)
    groups = np.asarray(groups)
    assert x.shape == (_B, _N), x.shape
    nc, forder, stats = _get_program(groups)

    # feature-major, permuted into first-use order
    xt = np.ascontiguousarray(x.T[forder].astype(np.float32, copy=False))  # [32, B]
    in_maps = [
        {"xt": np.ascontiguousarray(xt[:, c * _R : (c + 1) * _R])} for c in range(_NC)
    ]
    res = bass_utils.run_bass_kernel_spmd(
        nc, in_maps, core_ids=list(range(_NC)), trace=trace
    )
    y = (
        np.concatenate([np.asarray(res.results[c]["out"]) for c in range(_NC)])
        .astype(np.float32)
        .reshape(_B, 1)
    )
    return y, res, stats


def kernel(x, groups):
    y, _res, _stats = run(x, groups, trace=False)
    return y


# revision 6
# speedup vs baseline: 1.5520x; 1.1483x over previous
"""Trainium2 Bass kernel for segment_reduce (max over groups of min within group).

reference semantics:
    mins = min(x[:, groups], axis=-1)   # [B, G]
    out  = max(mins, axis=1, keepdims=True)  # [B, 1]

Strategy:
  - Pure data parallel: 8 NeuronCores, each handles B/8 = 131072 rows.
  - Host side: transpose x to feature-major [32, B] so each feature's column
    loads as a unit-stride SBUF tile [128, 1024] (rows spread over
    partition x free dims). DMA casts fp32 -> fp16 in flight (SWDGE).
  - Device side: the whole reduction is an elementwise min/max dataflow over
    32 feature tiles. The group structure is known at compile time (groups is
    tiny), so we bake an optimized op schedule:
      * drop dominated groups (superset feature-sets can never win the max)
      * factor via the distributive lattice law
            max_i min(f, A_i) = min(f, max_i A_i)
        building a 2-level trie (pivot f -> second pivot g -> max over third
        features), which cuts tensor ops to ~(G-1) + #f-nodes + #fg-nodes.
      * co-design the feature load order and the op order with a greedy
        dataflow simulation so compute starts with the first DMA chunk and
        stays fed while features stream in.
    All rewrites are exact in the (min,max) lattice. fp16 tensor_tensor runs
    in the DVE 2x perf mode; min/max are selections so the only rounding is
    the initial fp32->fp16 cast (rel err <= 2^-11).
  - Store: DMA casts fp16 -> fp32 on the way out.
"""

import os
from collections import Counter, OrderedDict

import numpy as np

_B, _N, _G = 1048576, 32, 64
_NC = 8
_R = _B // _NC  # rows per core
_P = 128
_W = _R // _P  # free-dim width per feature tile

# per-DMA feature counts: small first chunks so compute starts early
_CHUNKS = [1, 1] + [2] * 15
assert sum(_CHUNKS) == _N

_DT_NAME = os.environ.get("KERNEL_DT", "float16")

_prog_cache: dict = {}


def _dedup(groups: np.ndarray):
    """Sorted-distinct tuples, dominated groups removed."""
    gs = [tuple(sorted(set(int(v) for v in g))) for g in groups]
    sets = [frozenset(g) for g in gs]
    kept = []
    for i, si in enumerate(sets):
        dominated = False
        for j, sj in enumerate(sets):
            if i == j:
                continue
            # a proper-subset group has min >= ours, so ours never wins the max
            if sj < si or (sj == si and j < i):
                dominated = True
                break
        if not dominated:
            kept.append(gs[i])
    return kept


def _build_trie(kept):
    """Greedy 2-level factoring.

    Returns (trie, singles): trie is OrderedDict f -> OrderedDict(g -> [thirds])
    covering all groups of size >= 2; singles lists size-1 groups' features.
    """
    singles = [g[0] for g in kept if len(g) == 1]
    todo = [g for g in kept if len(g) >= 2]
    trie = OrderedDict()
    while todo:
        cnt = Counter(f for g in todo for f in g)
        f = max(cnt, key=lambda k: (cnt[k], -k))
        cluster = [g for g in todo if f in g]
        todo = [g for g in todo if f not in g]
        rests = [tuple(v for v in g if v != f) for g in cluster]
        sub = OrderedDict()
        while rests:
            c2 = Counter(v for r in rests for v in r)
            g2 = max(c2, key=lambda k: (c2[k], -k))
            mine = [r for r in rests if g2 in r]
            rests = [r for r in rests if g2 not in r]
            thirds = []
            for r in mine:
                rem = tuple(v for v in r if v != g2)
                thirds.append(rem[0] if rem else None)
            sub[g2] = thirds
        trie[f] = sub
    return trie, singles


class _Sub:
    __slots__ = ("g2", "thirds_left", "chain", "value", "n_chain")

    def __init__(self, g2, thirds):
        self.g2 = g2
        self.thirds_left = list(thirds)  # real third features not yet consumed
        self.chain = None  # buf holding the running max of thirds
        self.n_chain = len(thirds)
        self.value = None  # final sub value: ("buf",i) or ("feat",g2)


class _Cluster:
    __slots__ = ("f", "subs", "vacc", "n_pending", "closed")

    def __init__(self, f, sub):
        self.f = f
        self.subs = []
        for g2, thirds in sub.items():
            real = [t for t in thirds if t is not None]
            if real and len(real) != len(thirds):
                raise AssertionError("dominated group survived dedup")
            self.subs.append(_Sub(g2, real))
        self.vacc = None  # running max over sub values
        self.n_pending = len(self.subs)  # subs not yet joined into vacc
        self.closed = False


def _coschedule(trie, singles):
    """Greedy co-design of feature load order and op order.

    Emits ops as soon as their operands' features are 'loaded'; when no op is
    ready, loads the feature that unlocks the most work. Returns
    (ops, forder, max_live_bufs).
    """
    clusters = [_Cluster(f, sub) for f, sub in trie.items()]
    singles_left = list(singles)

    ops = []
    nbuf = [0]
    live = [0]
    max_live = [0]

    def newbuf():
        nbuf[0] += 1
        live[0] += 1
        max_live[0] = max(max_live[0], live[0])
        return ("buf", nbuf[0])

    def freebuf(v):
        if v[0] == "buf":
            live[0] -= 1

    loaded = set()
    forder = []
    acc = ("acc",)
    acc_started = [False]

    def join_acc(v):
        if not acc_started[0]:
            if v[0] == "feat":
                ops.append(("copy", acc, v))
            else:
                # value is in a buf: fold the final op into acc instead when
                # possible -- handled by callers via direct dst. Fallback:
                ops.append(("copy", acc, v))
            acc_started[0] = True
        else:
            ops.append(("max", acc, acc, v))
        freebuf(v)

    def emit_one():
        """Emit one ready op if any. Returns True if emitted/progressed."""
        # 0. close a cluster (min with pivot + acc join)
        for c in clusters:
            if c.closed or c.n_pending or c.f not in loaded:
                continue
            v = c.vacc
            if v is None:
                raise AssertionError("cluster with no subs")
            if not acc_started[0]:
                ops.append(("min", acc, ("feat", c.f), v))
                acc_started[0] = True
                freebuf(v)
            elif v[0] == "buf":
                ops.append(("min", v, ("feat", c.f), v))
                ops.append(("max", acc, acc, v))
                freebuf(v)
            else:
                b = newbuf()
                ops.append(("min", b, ("feat", c.f), v))
                ops.append(("max", acc, acc, b))
                freebuf(b)
            c.closed = True
            return True
        # 1. finalize a sub (min with g2) / realize 2-group values
        for c in clusters:
            if c.closed:
                continue
            for s in c.subs:
                if s.value is not None:
                    continue
                if s.n_chain == 0:
                    if s.g2 in loaded:
                        s.value = ("feat", s.g2)
                        return True
                    continue
                if not s.thirds_left and s.chain is not None and s.g2 in loaded:
                    ops.append(("min", s.chain, ("feat", s.g2), s.chain))
                    s.value = s.chain
                    return True
        # 2. join a sub value into the cluster accumulator
        for c in clusters:
            if c.closed:
                continue
            for s in c.subs:
                if s.value is None or s.value[0] == "joined":
                    continue
                if c.vacc is None:
                    c.vacc = s.value
                    s.value = ("joined",)
                    c.n_pending -= 1
                    return True
                if c.vacc[0] == "buf":
                    ops.append(("max", c.vacc, c.vacc, s.value))
                    freebuf(s.value)
                elif s.value[0] == "buf":
                    ops.append(("max", s.value, c.vacc, s.value))
                    c.vacc = s.value
                else:
                    b = newbuf()
                    ops.append(("max", b, c.vacc, s.value))
                    c.vacc = b
                s.value = ("joined",)
                c.n_pending -= 1
                return True
        # 3. join a loaded single
        for i, f in enumerate(singles_left):
            if f in loaded and acc_started[0]:
                ops.append(("max", acc, acc, ("feat", f)))
                singles_left.pop(i)
                return True
            if f in loaded and not acc_started[0]:
                ops.append(("copy", acc, ("feat", f)))
                acc_started[0] = True
                singles_left.pop(i)
                return True
        # 4. extend a chain with a loaded third
        for c in clusters:
            if c.closed:
                continue
            for s in c.subs:
                if s.value is not None or s.n_chain == 0:
                    continue
                have = [t for t in s.thirds_left if t in loaded]
                if s.chain is None:
                    if len(s.thirds_left) == 1 and have:
                        # single third: chain is just the feature; the min
                        # with g2 consumes it directly
                        t = have[0]
                        if s.g2 in loaded:
                            b = newbuf()
                            ops.append(("min", b, ("feat", s.g2), ("feat", t)))
                            s.thirds_left.remove(t)
                            s.value = b
                            return True
                        continue
                    if len(have) >= 2:
                        b = newbuf()
                        ops.append(("max", b, ("feat", have[0]), ("feat", have[1])))
                        s.thirds_left.remove(have[0])
                        s.thirds_left.remove(have[1])
                        s.chain = b
                        return True
                    continue
                if have:
                    ops.append(("max", s.chain, s.chain, ("feat", have[0])))
                    s.thirds_left.remove(have[0])
                    return True
        return False

    def unlock_score(f):
        """How much work loading f unlocks right now (cheap heuristic)."""
        sc = 0
        for c in clusters:
            if c.closed:
                continue
            if f == c.f and c.n_pending == 0:
                sc += 30  # closes a cluster immediately
            for s in c.subs:
                if s.value is not None:
                    continue
                if f == s.g2:
                    if s.n_chain == 0:
                        sc += 6
                    elif not s.thirds_left and s.chain is not None:
                        sc += 20
                    elif len(s.thirds_left) == 1 and s.thirds_left[0] in loaded:
                        sc += 15
                    else:
                        sc += 2
                if f in s.thirds_left:
                    have = sum(1 for t in s.thirds_left if t in loaded)
                    if s.chain is not None or have >= 1:
                        sc += 10
                    else:
                        sc += 3
        for s1 in singles_left:
            if f == s1:
                sc += 8
        return sc

    all_feats = sorted(
        {c.f for c in clusters}
        | {s.g2 for c in clusters for s in c.subs}
        | {t for c in clusters for s in c.subs for t in s.thirds_left}
        | set(singles_left)
    )
    while True:
        if emit_one():
            continue
        remaining = [f for f in all_feats if f not in loaded]
        if not remaining:
            break
        best = max(remaining, key=lambda f: (unlock_score(f), -f))
        loaded.add(best)
        forder.append(best)

    # drain: everything loaded now; emit the rest
    while emit_one():
        pass
    assert not singles_left and all(c.closed for c in clusters), "incomplete"

    for f in range(_N):
        if f not in loaded:
            forder.append(f)
    n_tt = sum(1 for o in ops if o[0] in ("min", "max"))
    return ops, forder, n_tt, max_live[0]


def _simulate(ops, groups, x):
    """Numpy execution of the op list (for self-checks)."""
    bufs = {}

    def val(v):
        if v[0] == "feat":
            return x[:, v[1]]
        return bufs[v]

    for op in ops:
        if op[0] == "copy":
            bufs[op[1]] = val(op[2]).copy()
        elif op[0] == "min":
            bufs[op[1]] = np.minimum(val(op[2]), val(op[3]))
        else:
            bufs[op[1]] = np.maximum(val(op[2]), val(op[3]))
    return bufs[("acc",)]


def _make_plan(groups: np.ndarray):
    kept = _dedup(groups)
    trie, singles = _build_trie(kept)
    ops, forder, n_tt, max_live = _coschedule(trie, singles)
    # self-check the schedule against brute force on random data
    xs = np.random.default_rng(0).standard_normal((256, _N)).astype(np.float32)
    want = xs[:, np.asarray(groups)].min(-1).max(1)
    got = _simulate(ops, groups, xs)
    assert np.array_equal(got, want), "schedule self-check failed"
    stats = {
        "n_groups_kept": len(kept),
        "n_f_nodes": len(trie),
        "n_fg_nodes": sum(len(s) for s in trie.values()),
        "n_tt_ops": n_tt,
        "max_live_bufs": max_live,
    }
    return ops, forder, stats


def _build_program(groups_tuple, dt_name):
    from concourse import bacc, mybir
    from concourse.tile import TileContext

    groups = np.array(groups_tuple, dtype=np.int64)
    ops, forder, stats = _make_plan(groups)
    fpos = {f: i for i, f in enumerate(forder)}  # feature id -> storage slot

    DT = getattr(mybir.dt, dt_name)
    cast = dt_name != "float32"

    nc = bacc.Bacc("TRN2", debug=False, enable_asserts=False, num_devices=_NC)
    xt = nc.dram_tensor("xt", [_N, _R], mybir.dt.float32, kind="ExternalInput")
    out = nc.dram_tensor("out", [_R], mybir.dt.float32, kind="ExternalOutput")

    with TileContext(nc) as tc:
        with (
            tc.tile_pool(name="feats", bufs=1) as fpool,
            tc.tile_pool(name="work", bufs=max(8, stats["max_live_bufs"] + 2)) as wpool,
        ):
            chunk_tiles = []
            slot2chunk = {}
            s0 = 0
            for ci, csz in enumerate(_CHUNKS):
                t = fpool.tile([_P, csz, _W], DT, tag=f"chunk{ci}", name=f"chunk{ci}")
                dma = nc.gpsimd if cast else nc.sync
                dma.dma_start(
                    out=t[:],
                    in_=xt[s0 : s0 + csz, :].rearrange("f (p w) -> p f w", p=_P),
                )
                chunk_tiles.append(t)
                for k in range(csz):
                    slot2chunk[s0 + k] = (ci, k)
                s0 += csz

            def feat_ap(f):
                ci, k = slot2chunk[fpos[f]]
                return chunk_tiles[ci][:, k, :]

            buf_tiles = {}
            acc_tile = wpool.tile([_P, _W], DT, tag="acc", name="acc")

            def val_ap(v):
                if v[0] == "feat":
                    return feat_ap(v[1])
                if v[0] == "acc":
                    return acc_tile[:]
                return buf_tiles[v][:]

            def dst_ap(v):
                if v[0] == "acc":
                    return acc_tile[:]
                if v not in buf_tiles:
                    buf_tiles[v] = wpool.tile(
                        [_P, _W], DT, tag="u", name=f"u{v[1]}"
                    )
                return buf_tiles[v][:]

            alu = {"min": mybir.AluOpType.min, "max": mybir.AluOpType.max}
            for op in ops:
                if op[0] == "copy":
                    nc.vector.tensor_copy(out=dst_ap(op[1]), in_=val_ap(op[2]))
                else:
                    nc.vector.tensor_tensor(
                        dst_ap(op[1]), val_ap(op[2]), val_ap(op[3]), alu[op[0]]
                    )

            st = nc.gpsimd if cast else nc.sync
            st.dma_start(out=out.rearrange("(p w) -> p w", p=_P), in_=acc_tile[:])

    nc.compile()
    return nc, forder, stats


def _get_program(groups: np.ndarray):
    key = (tuple(map(tuple, np.asarray(groups).tolist())), _DT_NAME)
    if key not in _prog_cache:
        _prog_cache[key] = _build_program(key[0], _DT_NAME)
    return _prog_cache[key]


def run(x, groups, trace=False):
    """Returns (out [B,1] fp32, BassKernelResults, stats)."""
    from concourse import bass_utils

    x = np.asarray(x)
    groups = np.asarray(groups)
    assert x.shape == (_B, _N), x.shape
    nc, forder, stats = _get_program(groups)

    # feature-major, permuted into load order
    xt = np.ascontiguousarray(x.T[forder].astype(np.float32, copy=False))
    in_maps = [
        {"xt": np.ascontiguousarray(xt[:, c * _R : (c + 1) * _R])} for c in range(_NC)
    ]
    res = bass_utils.run_bass_kernel_spmd(
        nc, in_maps, core_ids=list(range(_NC)), trace=trace
    )
    y = (
        np.concatenate([np.asarray(res.results[c]["out"]) for c in range(_NC)])
        .astype(np.float32)
        .reshape(_B, 1)
    )
    return y, res, stats


def kernel(x, groups):
    y, _res, _stats = run(x, groups, trace=False)
    return y
